# revision 16
# baseline (speedup 1.0000x reference)
"""Block-sparse MoE (sparse expert-parallel) Trainium2 kernel.

Problem: nn_BlockSparseMoE_15882789061249
  T=1024 tokens, H=2048 hidden, F=1408 intermediate, E=16 experts, top_k=6.

Strategy (8 NeuronCores, SPMD single program):
  - Expert parallel: core c owns experts {2c, 2c+1}; weights sharded by
    expert on the host, x and the gate replicated (x is 8 MB vs 554 MB of
    weights, so replicating x beats an all-to-all dispatch at this scale).
  - Host permutes gate columns per core so each core's experts are route
    columns 0/1 -> one SPMD program for all cores.
  - fp32 router on device: logits -> exp -> top-6 (DVE max8/match_replace)
    -> renormalized weights.
  - Sparse dispatch: per local expert, token positions are computed with
    triangular-matmul prefix sums; token rows (augmented with their route
    weight and token id) are scattered into a capacity-512 buffer via
    indirect DMA, transposed on the PE, and only the gathered tokens run
    through the expert MLP (bf16). Outputs are scaled and scatter-added
    back to a DRAM partial by token id, then an 8-core ReduceScatter
    produces each core's 128-token shard; the host concatenates shards.
"""

import numpy as np

T, H, F, E = 1024, 2048, 1408, 16
NCORES = 8
TOPK = 6
CAP = 512  # per-expert token capacity (seed-traffic max is ~418; 6.5 sigma)

_CACHE = {}


def build_moe_nc(t, h, f, e, n_cores, topk=6, cap=512):
    import concourse.bacc as bacc
    import concourse.mybir as mybir
    import concourse.tile as tile
    from concourse.bass import IndirectOffsetOnAxis

    f32 = mybir.dt.float32
    bf16 = mybir.dt.bfloat16
    u32 = mybir.dt.uint32
    AF = mybir.ActivationFunctionType
    Alu = mybir.AluOpType
    X = mybir.AxisListType.X

    epc = e // n_cores
    assert epc == 2, "program is specialized for 2 experts/core"
    kh = h // 128
    kf = f // 128
    mt = t // 128
    tsh = t // n_cores
    ct_n = cap // 128          # capacity tiles
    ha = h + 4                 # aug row: x | wt0 wt1 hi lo
    nh = [(i, min(512, h - i)) for i in range(0, h, 512)]
    BIG = 4.0e6

    nc = bacc.Bacc("TRN2", target_bir_lowering=False, debug=False,
                   num_devices=n_cores)

    xT = nc.dram_tensor("xT", [h, t], f32, kind="ExternalInput")
    xr = nc.dram_tensor("xr", [t, h], bf16, kind="ExternalInput")
    thl = nc.dram_tensor("thl", [t, 2], bf16, kind="ExternalInput")
    gwT = nc.dram_tensor("gwT", [h, e], f32, kind="ExternalInput")
    wv1t = nc.dram_tensor("wv1t", [epc, kh, 2 * kf, 128, 128], bf16,
                          kind="ExternalInput")
    w2t = nc.dram_tensor("w2t", [epc, kf, 128, h], bf16, kind="ExternalInput")
    ident = nc.dram_tensor("ident", [128, 128], f32, kind="ExternalInput")
    ones = nc.dram_tensor("ones", [128, 128], f32, kind="ExternalInput")
    sut = nc.dram_tensor("sut", [128, 128], f32, kind="ExternalInput")
    out_sh = nc.dram_tensor("out_shard", [tsh, h], f32, kind="ExternalOutput")

    xg = [nc.dram_tensor(f"xg{le}", [cap, ha], bf16) for le in range(epc)]
    partial = nc.dram_tensor("partial", [t, h], f32)
    rs_out = nc.dram_tensor("rs_out", [tsh, h], f32)

    with tile.TileContext(nc) as tc:
        with tc.tile_pool(name="persist", bufs=1) as pp:
            gw = pp.tile([128, kh * e], f32, tag="gw")
            ids = pp.tile([128, 128], f32, tag="ids")
            idb = pp.tile([128, 128], bf16, tag="idb")
            onesb = pp.tile([128, 128], f32, tag="onesb")
            sub = pp.tile([128, 128], f32, tag="sub")
            route = pp.tile([128, mt * e], f32, tag="route")
            masks = pp.tile([128, mt * epc], f32, tag="masks")
            lg = pp.tile([128, t], f32, tag="lg")
            xgt = pp.tile([128, epc * kh * cap], bf16, tag="xgt")
            actg = pp.tile([128, epc * kf * cap], bf16, tag="actg")
            wtc = pp.tile([128, epc * ct_n], f32, tag="wtc")
            tokc = pp.tile([128, epc * ct_n], u32, tag="tokc")
            zf = pp.tile([128, h], f32, tag="zf")
            zb = pp.tile([128, ha], bf16, tag="zb")

            nc.sync.dma_start(out=ids[:], in_=ident[:, :])
            nc.vector.tensor_copy(out=idb[:], in_=ids[:])
            nc.sync.dma_start(out=onesb[:], in_=ones[:, :])
            nc.sync.dma_start(out=sub[:], in_=sut[:, :])
            nc.vector.memset(zf[:], 0.0)
            nc.vector.memset(zb[:], 0.0)
            for k in range(kh):
                nc.sync.dma_start(out=gw[:, k * e:(k + 1) * e],
                                  in_=gwT[k * 128:(k + 1) * 128, :])
            # zero the partial accumulator and the aug tail columns
            for tt in range(mt):
                nc.sync.dma_start(out=partial[tt * 128:(tt + 1) * 128, :],
                                  in_=zf[:])
            for le in range(epc):
                for ci in range(ct_n):
                    nc.sync.dma_start(
                        out=xg[le][ci * 128:(ci + 1) * 128, :],
                        in_=zb[:])
            # indirect-DMA writes are region-tracked by their static shape
            # only; fence so the zero fills land before any scatter.
            tc.strict_bb_all_engine_barrier()

            # ---- router logits [e, t] ----
            with (tc.tile_pool(name="xload", bufs=3) as pxl,
                  tc.tile_pool(name="psr", bufs=1, space="PSUM") as ppr):
                psl = ppr.tile([128, t], f32, tag="psl")
                for k in range(kh):
                    xf = pxl.tile([128, t], f32, tag="xf")
                    nc.sync.dma_start(out=xf[:],
                                      in_=xT[k * 128:(k + 1) * 128, :])
                    for n0 in range(0, t, 512):
                        nsz = min(512, t - n0)
                        nc.tensor.matmul(
                            psl[:e, n0:n0 + nsz],
                            lhsT=gw[:, k * e:(k + 1) * e],
                            rhs=xf[:, n0:n0 + nsz],
                            start=(k == 0), stop=(k == kh - 1))
                nc.vector.tensor_copy(out=lg[:e, :], in_=psl[:e, :])

            # ---- router: top-k renormalized weights per token tile ----
            with (tc.tile_pool(name="rt", bufs=2) as prt,
                  tc.tile_pool(name="pst", bufs=2, space="PSUM") as ppt):
                for tt in range(mt):
                    ptile = ppt.tile([128, e], f32, tag="ltr")
                    nc.tensor.transpose(ptile[:, :],
                                        lg[:e, tt * 128:(tt + 1) * 128],
                                        ids[:e, :e])
                    mx = prt.tile([128, 1], f32, tag="mx")
                    nc.vector.reduce_max(out=mx[:], in_=ptile[:, :], axis=X)
                    nm = prt.tile([128, 1], f32, tag="nm")
                    nc.vector.tensor_scalar_mul(nm[:], mx[:], -1.0)
                    ev = prt.tile([128, e], f32, tag="ev")
                    nc.scalar.activation(ev[:], ptile[:, :], AF.Exp,
                                         bias=nm[:], scale=1.0)
                    t8 = prt.tile([128, 8], f32, tag="t8")
                    nc.vector.max(out=t8[:], in_=ev[:])
                    if topk < 8:
                        nc.vector.memset(t8[:, topk:], 0.0)
                    zap = prt.tile([128, e], f32, tag="zap")
                    nc.vector.match_replace(out=zap[:], in_to_replace=t8[:],
                                            in_values=ev[:], imm_value=0.0)
                    msk = prt.tile([128, e], f32, tag="msk")
                    nc.vector.tensor_sub(msk[:], ev[:], zap[:])
                    dn = prt.tile([128, 1], f32, tag="dn")
                    nc.vector.reduce_sum(out=dn[:], in_=msk[:], axis=X)
                    iv = prt.tile([128, 1], f32, tag="iv")
                    nc.vector.reciprocal(iv[:], dn[:])
                    nc.vector.tensor_scalar_mul(
                        route[:, tt * e:(tt + 1) * e], msk[:], iv[:])
                    # 0/1 mask for this core's two experts
                    nc.vector.tensor_scalar(
                        masks[:, tt * epc:(tt + 1) * epc],
                        route[:, tt * e:tt * e + epc],
                        0.0, scalar2=None, op0=Alu.is_gt)

            # ---- dispatch: positions via prefix-sum matmuls + scatter ----
            with (tc.tile_pool(name="dsp", bufs=3) as pds,
                  tc.tile_pool(name="psd", bufs=2, space="PSUM") as ppd):
                for tt in range(mt):
                    aug = pds.tile([128, ha], bf16, tag="aug")
                    nc.sync.dma_start(
                        out=aug[:, :h], in_=xr[tt * 128:(tt + 1) * 128, :])
                    nc.sync.dma_start(
                        out=aug[:, h + 2:h + 4],
                        in_=thl[tt * 128:(tt + 1) * 128, :])
                    nc.vector.tensor_copy(
                        out=aug[:, h:h + epc],
                        in_=route[:, tt * e:tt * e + epc])
                    pos = ppd.tile([128, epc], f32, tag="pos")
                    for j in range(tt):
                        nc.tensor.matmul(
                            pos[:, :], lhsT=onesb[:],
                            rhs=masks[:, j * epc:(j + 1) * epc],
                            start=(j == 0), stop=False)
                    nc.tensor.matmul(
                        pos[:, :], lhsT=sub[:],
                        rhs=masks[:, tt * epc:(tt + 1) * epc],
                        start=(tt == 0), stop=True)
                    # unrouted tokens -> position past capacity (skipped)
                    um = pds.tile([128, epc], f32, tag="um")
                    nc.vector.tensor_scalar(
                        um[:], masks[:, tt * epc:(tt + 1) * epc],
                        0.0, scalar2=BIG, op0=Alu.is_le, op1=Alu.mult)
                    pf = pds.tile([128, epc], f32, tag="pf")
                    nc.vector.tensor_add(pf[:], pos[:, :], um[:])
                    pu32 = pds.tile([128, epc], u32, tag="pu32")
                    nc.vector.tensor_copy(out=pu32[:], in_=pf[:])
                    for le in range(epc):
                        nc.gpsimd.indirect_dma_start(
                            out=xg[le][:, :],
                            out_offset=IndirectOffsetOnAxis(
                                ap=pu32[:, le:le + 1], axis=0),
                            in_=aug[:],
                            in_offset=None,
                            bounds_check=cap - 1,
                            oob_is_err=False)

            # all dispatch scatters must land before any xg row is read
            tc.strict_bb_all_engine_barrier()

            # ---- gather + PE transpose to [h, slot] layout ----
            with (tc.tile_pool(name="gth", bufs=3) as pgt,
                  tc.tile_pool(name="psg", bufs=3, space="PSUM") as ppg):
                for le in range(epc):
                    for ci in range(ct_n):
                        row = pgt.tile([128, ha], bf16, tag="row")
                        nc.sync.dma_start(
                            out=row[:],
                            in_=xg[le][ci * 128:(ci + 1) * 128, :])
                        cx = le * ct_n + ci
                        nc.vector.tensor_copy(out=wtc[:, cx:cx + 1],
                                              in_=row[:, h + le:h + le + 1])
                        tf = pgt.tile([128, 1], f32, tag="tf")
                        nc.vector.tensor_scalar(
                            tf[:], row[:, h + 2:h + 3], 32.0, scalar2=None,
                            op0=Alu.mult)
                        tf2 = pgt.tile([128, 1], f32, tag="tf2")
                        nc.vector.tensor_add(tf2[:], tf[:],
                                             row[:, h + 3:h + 4])
                        # pad slots (weight == 0) -> OOB token id so the
                        # output scatter skips them entirely
                        pb = pgt.tile([128, 1], f32, tag="pb")
                        nc.vector.tensor_scalar(
                            pb[:], wtc[:, cx:cx + 1], 0.0, scalar2=BIG,
                            op0=Alu.is_le, op1=Alu.mult)
                        tf3 = pgt.tile([128, 1], f32, tag="tf3")
                        nc.vector.tensor_add(tf3[:], tf2[:], pb[:])
                        nc.vector.tensor_copy(out=tokc[:, cx:cx + 1],
                                              in_=tf3[:])
                        for k in range(kh):
                            ptp = ppg.tile([128, 128], bf16, tag="ptp")
                            nc.tensor.transpose(
                                ptp[:, :], row[:, k * 128:(k + 1) * 128],
                                idb[:])
                            nc.vector.tensor_copy(
                                out=xgt[:, (le * kh + k) * cap + ci * 128:
                                        (le * kh + k) * cap + ci * 128 + 128],
                                in_=ptp[:, :])

            # ---- phase A: actg[f, slot] = silu(g) * u ----
            with (tc.tile_pool(name="wv", bufs=6) as pwv,
                  tc.tile_pool(name="sg", bufs=2) as psg,
                  tc.tile_pool(name="psa", bufs=4, space="PSUM") as ppa):
                for le in range(epc):
                    for m in range(kf):
                        pg = ppa.tile([128, cap], f32, tag="pg")
                        pu = ppa.tile([128, cap], f32, tag="pu")
                        for k in range(kh):
                            wg = pwv.tile([128, 128], bf16, tag="wg")
                            nc.sync.dma_start(out=wg[:], in_=wv1t[le, k, m])
                            wu = pwv.tile([128, 128], bf16, tag="wu")
                            nc.sync.dma_start(out=wu[:],
                                              in_=wv1t[le, k, m + kf])
                            rh = xgt[:, (le * kh + k) * cap:
                                     (le * kh + k) * cap + cap]
                            nc.tensor.matmul(pg[:, :], lhsT=wg[:], rhs=rh,
                                             start=(k == 0),
                                             stop=(k == kh - 1))
                            nc.tensor.matmul(pu[:, :], lhsT=wu[:], rhs=rh,
                                             start=(k == 0),
                                             stop=(k == kh - 1))
                        sgm = psg.tile([128, cap], bf16, tag="sgm")
                        nc.scalar.activation(sgm[:], pg[:], AF.Sigmoid)
                        sg = psg.tile([128, cap], bf16, tag="sg")
                        nc.vector.tensor_mul(out=sg[:], in0=sgm[:], in1=pg[:])
                        ai = (le * kf + m) * cap
                        nc.vector.tensor_mul(out=actg[:, ai:ai + cap],
                                             in0=sg[:], in1=pu[:])

            # ---- phase B: y = actg @ w2T, scale, scatter-add ----
            with (tc.tile_pool(name="w2p", bufs=kf + 3) as pw2,
                  tc.tile_pool(name="sc", bufs=2) as psc,
                  tc.tile_pool(name="psb", bufs=2, space="PSUM") as ppb):
                for le in range(epc):
                    w2ks = []
                    for k in range(kf):
                        w2k = pw2.tile([128, h], bf16, tag="w2k")
                        nc.sync.dma_start(out=w2k[:], in_=w2t[le, k])
                        w2ks.append(w2k)
                    for ci in range(ct_n):
                        cx = le * ct_n + ci
                        py = ppb.tile([128, h], f32, tag="py")
                        for k in range(kf):
                            ai = (le * kf + k) * cap + ci * 128
                            for (n0, nsz) in nh:
                                nc.tensor.matmul(
                                    py[:, n0:n0 + nsz],
                                    lhsT=actg[:, ai:ai + 128],
                                    rhs=w2ks[k][:, n0:n0 + nsz],
                                    start=(k == 0), stop=(k == kf - 1))
                        sc = psc.tile([128, h], f32, tag="sc")
                        nc.vector.tensor_scalar_mul(sc[:], py[:, :],
                                                    wtc[:, cx:cx + 1])
                        nc.gpsimd.indirect_dma_start(
                            out=partial[:, :],
                            out_offset=IndirectOffsetOnAxis(
                                ap=tokc[:, cx:cx + 1], axis=0),
                            in_=sc[:],
                            in_offset=None,
                            bounds_check=t - 1,
                            oob_is_err=False,
                            compute_op=Alu.add)

            # all scatter-adds must land before the collective reads partial
            tc.strict_bb_all_engine_barrier()

            # ---- cross-core reduce-scatter + shard output ----
            nc.gpsimd.collective_compute(
                "ReduceScatter", Alu.add,
                replica_groups=[list(range(n_cores))],
                ins=[partial.ap().opt()],
                outs=[rs_out.ap().opt()],
            )
            nc.sync.dma_start(out=out_sh[:, :], in_=rs_out[:, :])

    nc.compile()
    return nc


def prep_inputs(x, gate_w, wv1, w2, t, h, f, e, n_cores):
    """Host-side shard/cast/tile. Returns per-core input maps."""
    import ml_dtypes
    bf16 = ml_dtypes.bfloat16

    epc = e // n_cores
    kh = h // 128
    kf = f // 128
    mf2 = 2 * f // 128

    xT = np.ascontiguousarray(x.T).astype(np.float32)        # [h, t]
    xr = np.ascontiguousarray(x).astype(bf16)                # [t, h]
    toks = np.arange(t)
    thl = np.stack([toks // 32, toks % 32], axis=1).astype(bf16)
    ident = np.eye(128, dtype=np.float32)
    ones = np.ones((128, 128), dtype=np.float32)
    # sut[q, p] = 1 if q < p  (strict upper)  -> (SU.T @ m)[p] = sum_{q<p} m[q]
    sut = np.triu(np.ones((128, 128), dtype=np.float32), k=1)

    in_maps = []
    for c in range(n_cores):
        own = list(range(c * epc, (c + 1) * epc))
        rest = [i for i in range(e) if i not in own]
        perm = own + rest
        gwT = np.ascontiguousarray(gate_w[perm].T).astype(np.float32)

        wl = wv1[own]                                        # [epc, 2f, h]
        wv1tc = np.ascontiguousarray(
            wl.transpose(0, 2, 1)
              .reshape(epc, kh, 128, mf2, 128)
              .transpose(0, 1, 3, 2, 4)).astype(bf16)

        w2l = w2[own]                                        # [epc, h, f]
        w2tc = np.ascontiguousarray(
            w2l.transpose(0, 2, 1)
               .reshape(epc, kf, 128, h)).astype(bf16)

        in_maps.append({
            "xT": xT, "xr": xr, "thl": thl, "gwT": gwT,
            "wv1t": wv1tc, "w2t": w2tc,
            "ident": ident, "ones": ones, "sut": sut,
        })
    return in_maps


def kernel(x, gate_w, wv1, w2, top_k):
    from concourse.bass_utils import run_bass_kernel_spmd

    assert int(top_k) == TOPK
    x = np.asarray(x, dtype=np.float32)
    gate_w = np.asarray(gate_w, dtype=np.float32)
    wv1 = np.asarray(wv1, dtype=np.float32)
    w2 = np.asarray(w2, dtype=np.float32)

    key = (T, H, F, E, NCORES)
    if key not in _CACHE:
        _CACHE[key] = build_moe_nc(T, H, F, E, NCORES, TOPK, CAP)
    nc = _CACHE[key]

    in_maps = prep_inputs(x, gate_w, wv1, w2, T, H, F, E, NCORES)
    res = run_bass_kernel_spmd(nc, in_maps, list(range(NCORES)))
    shards = [res.results[c]["out_shard"] for c in range(NCORES)]
    return np.concatenate(shards, axis=0).astype(np.float32)


# revision 17
# speedup vs baseline: 1.0021x; 1.0021x over previous
"""Block-sparse MoE (sparse expert-parallel) Trainium2 kernel.

Problem: nn_BlockSparseMoE_15882789061249
  T=1024 tokens, H=2048 hidden, F=1408 intermediate, E=16 experts, top_k=6.

Strategy (8 NeuronCores, SPMD single program):
  - Expert parallel: core c owns experts {2c, 2c+1}; weights sharded by
    expert on the host, x and the gate replicated (x is 8 MB vs 554 MB of
    weights, so replicating x beats an all-to-all dispatch at this scale).
  - Host permutes gate columns per core so each core's experts are route
    columns 0/1 -> one SPMD program for all cores.
  - fp32 router on device: logits -> exp -> top-6 (DVE max8/match_replace)
    -> renormalized weights.
  - Sparse dispatch: per local expert, token positions are computed with
    triangular-matmul prefix sums; token rows (augmented with their route
    weight and token id) are scattered into a capacity-512 buffer via
    indirect DMA, transposed on the PE, and only the gathered tokens run
    through the expert MLP (bf16). Outputs are scaled and scatter-added
    back to a DRAM partial by token id, then an 8-core ReduceScatter
    produces each core's 128-token shard; the host concatenates shards.
"""

import numpy as np

T, H, F, E = 1024, 2048, 1408, 16
NCORES = 8
TOPK = 6
CAP = 512  # per-expert token capacity (seed-traffic max is ~418; 6.5 sigma)

_CACHE = {}


def build_moe_nc(t, h, f, e, n_cores, topk=6, cap=512):
    import concourse.bacc as bacc
    import concourse.mybir as mybir
    import concourse.tile as tile
    from concourse.bass import IndirectOffsetOnAxis

    f32 = mybir.dt.float32
    bf16 = mybir.dt.bfloat16
    u32 = mybir.dt.uint32
    AF = mybir.ActivationFunctionType
    Alu = mybir.AluOpType
    X = mybir.AxisListType.X

    epc = e // n_cores
    assert epc == 2, "program is specialized for 2 experts/core"
    kh = h // 128
    kf = f // 128
    mt = t // 128
    tsh = t // n_cores
    ct_n = cap // 128          # capacity tiles
    ha = h + 4                 # aug row: x | wt0 wt1 hi lo
    nh = [(i, min(512, h - i)) for i in range(0, h, 512)]
    BIG = 4.0e6

    nc = bacc.Bacc("TRN2", target_bir_lowering=False, debug=False,
                   num_devices=n_cores)

    xT = nc.dram_tensor("xT", [h, t], f32, kind="ExternalInput")
    xr = nc.dram_tensor("xr", [t, h], bf16, kind="ExternalInput")
    thl = nc.dram_tensor("thl", [t, 2], bf16, kind="ExternalInput")
    gwT = nc.dram_tensor("gwT", [h, e], f32, kind="ExternalInput")
    wv1t = nc.dram_tensor("wv1t", [epc, kh, 2 * kf, 128, 128], bf16,
                          kind="ExternalInput")
    w2t = nc.dram_tensor("w2t", [epc, kf, 128, h], bf16, kind="ExternalInput")
    ident = nc.dram_tensor("ident", [128, 128], f32, kind="ExternalInput")
    ones = nc.dram_tensor("ones", [128, 128], f32, kind="ExternalInput")
    sut = nc.dram_tensor("sut", [128, 128], f32, kind="ExternalInput")
    out_sh = nc.dram_tensor("out_shard", [tsh, h], f32, kind="ExternalOutput")

    xg = [nc.dram_tensor(f"xg{le}", [cap, ha], bf16) for le in range(epc)]
    partial = nc.dram_tensor("partial", [t, h], f32)
    rs_out = nc.dram_tensor("rs_out", [tsh, h], f32)

    with tile.TileContext(nc) as tc:
        with tc.tile_pool(name="persist", bufs=1) as pp:
            gw = pp.tile([128, kh * e], f32, tag="gw")
            ids = pp.tile([128, 128], f32, tag="ids")
            idb = pp.tile([128, 128], bf16, tag="idb")
            onesb = pp.tile([128, 128], f32, tag="onesb")
            sub = pp.tile([128, 128], f32, tag="sub")
            route = pp.tile([128, mt * e], f32, tag="route")
            masks = pp.tile([128, mt * epc], f32, tag="masks")
            lg = pp.tile([128, t], f32, tag="lg")
            xgt = pp.tile([128, epc * kh * cap], bf16, tag="xgt")
            actg = pp.tile([128, epc * kf * cap], bf16, tag="actg")
            wtc = pp.tile([128, epc * ct_n], f32, tag="wtc")
            tokc = pp.tile([128, epc * ct_n], u32, tag="tokc")
            zf = pp.tile([128, h], f32, tag="zf")
            zb = pp.tile([128, ha], bf16, tag="zb")

            nc.sync.dma_start(out=ids[:], in_=ident[:, :])
            nc.vector.tensor_copy(out=idb[:], in_=ids[:])
            nc.sync.dma_start(out=onesb[:], in_=ones[:, :])
            nc.sync.dma_start(out=sub[:], in_=sut[:, :])
            nc.vector.memset(zf[:], 0.0)
            nc.vector.memset(zb[:], 0.0)
            for k in range(kh):
                nc.sync.dma_start(out=gw[:, k * e:(k + 1) * e],
                                  in_=gwT[k * 128:(k + 1) * 128, :])
            # zero the partial accumulator and the aug tail columns
            for tt in range(mt):
                nc.sync.dma_start(out=partial[tt * 128:(tt + 1) * 128, :],
                                  in_=zf[:])
            for le in range(epc):
                for ci in range(ct_n):
                    nc.sync.dma_start(
                        out=xg[le][ci * 128:(ci + 1) * 128, :],
                        in_=zb[:])
            # indirect-DMA writes are region-tracked by their static shape
            # only; fence so the zero fills land before any scatter.
            tc.strict_bb_all_engine_barrier()

            # ---- router logits [e, t] ----
            with (tc.tile_pool(name="xload", bufs=3) as pxl,
                  tc.tile_pool(name="psr", bufs=1, space="PSUM") as ppr):
                psl = ppr.tile([128, t], f32, tag="psl")
                for k in range(kh):
                    xf = pxl.tile([128, t], f32, tag="xf")
                    nc.sync.dma_start(out=xf[:],
                                      in_=xT[k * 128:(k + 1) * 128, :])
                    for n0 in range(0, t, 512):
                        nsz = min(512, t - n0)
                        nc.tensor.matmul(
                            psl[:e, n0:n0 + nsz],
                            lhsT=gw[:, k * e:(k + 1) * e],
                            rhs=xf[:, n0:n0 + nsz],
                            start=(k == 0), stop=(k == kh - 1))
                nc.vector.tensor_copy(out=lg[:e, :], in_=psl[:e, :])

            # ---- router: top-k renormalized weights per token tile ----
            with (tc.tile_pool(name="rt", bufs=2) as prt,
                  tc.tile_pool(name="pst", bufs=2, space="PSUM") as ppt):
                for tt in range(mt):
                    ptile = ppt.tile([128, e], f32, tag="ltr")
                    nc.tensor.transpose(ptile[:, :],
                                        lg[:e, tt * 128:(tt + 1) * 128],
                                        ids[:e, :e])
                    mx = prt.tile([128, 1], f32, tag="mx")
                    nc.vector.reduce_max(out=mx[:], in_=ptile[:, :], axis=X)
                    nm = prt.tile([128, 1], f32, tag="nm")
                    nc.vector.tensor_scalar_mul(nm[:], mx[:], -1.0)
                    ev = prt.tile([128, e], f32, tag="ev")
                    nc.scalar.activation(ev[:], ptile[:, :], AF.Exp,
                                         bias=nm[:], scale=1.0)
                    t8 = prt.tile([128, 8], f32, tag="t8")
                    nc.vector.max(out=t8[:], in_=ev[:])
                    if topk < 8:
                        nc.vector.memset(t8[:, topk:], 0.0)
                    zap = prt.tile([128, e], f32, tag="zap")
                    nc.vector.match_replace(out=zap[:], in_to_replace=t8[:],
                                            in_values=ev[:], imm_value=0.0)
                    msk = prt.tile([128, e], f32, tag="msk")
                    nc.vector.tensor_sub(msk[:], ev[:], zap[:])
                    dn = prt.tile([128, 1], f32, tag="dn")
                    nc.vector.reduce_sum(out=dn[:], in_=msk[:], axis=X)
                    iv = prt.tile([128, 1], f32, tag="iv")
                    nc.vector.reciprocal(iv[:], dn[:])
                    nc.vector.tensor_scalar_mul(
                        route[:, tt * e:(tt + 1) * e], msk[:], iv[:])
                    # 0/1 mask for this core's two experts
                    nc.vector.tensor_scalar(
                        masks[:, tt * epc:(tt + 1) * epc],
                        route[:, tt * e:tt * e + epc],
                        0.0, scalar2=None, op0=Alu.is_gt)

            # ---- dispatch: positions via prefix-sum matmuls + scatter ----
            with (tc.tile_pool(name="dsp", bufs=3) as pds,
                  tc.tile_pool(name="psd", bufs=2, space="PSUM") as ppd):
                for tt in range(mt):
                    aug = pds.tile([128, ha], bf16, tag="aug")
                    nc.sync.dma_start(
                        out=aug[:, :h], in_=xr[tt * 128:(tt + 1) * 128, :])
                    nc.sync.dma_start(
                        out=aug[:, h + 2:h + 4],
                        in_=thl[tt * 128:(tt + 1) * 128, :])
                    nc.vector.tensor_copy(
                        out=aug[:, h:h + epc],
                        in_=route[:, tt * e:tt * e + epc])
                    pos = ppd.tile([128, epc], f32, tag="pos")
                    for j in range(tt):
                        nc.tensor.matmul(
                            pos[:, :], lhsT=onesb[:],
                            rhs=masks[:, j * epc:(j + 1) * epc],
                            start=(j == 0), stop=False)
                    nc.tensor.matmul(
                        pos[:, :], lhsT=sub[:],
                        rhs=masks[:, tt * epc:(tt + 1) * epc],
                        start=(tt == 0), stop=True)
                    # unrouted tokens -> position past capacity (skipped)
                    um = pds.tile([128, epc], f32, tag="um")
                    nc.vector.tensor_scalar(
                        um[:], masks[:, tt * epc:(tt + 1) * epc],
                        0.0, scalar2=BIG, op0=Alu.is_le, op1=Alu.mult)
                    pf = pds.tile([128, epc], f32, tag="pf")
                    nc.vector.tensor_add(pf[:], pos[:, :], um[:])
                    pu32 = pds.tile([128, epc], u32, tag="pu32")
                    nc.vector.tensor_copy(out=pu32[:], in_=pf[:])
                    for le in range(epc):
                        nc.gpsimd.indirect_dma_start(
                            out=xg[le][:, :],
                            out_offset=IndirectOffsetOnAxis(
                                ap=pu32[:, le:le + 1], axis=0),
                            in_=aug[:],
                            in_offset=None,
                            bounds_check=cap - 1,
                            oob_is_err=False)

            # all dispatch scatters must land before any xg row is read
            tc.strict_bb_all_engine_barrier()

            # ---- gather + PE transpose to [h, slot] layout ----
            with (tc.tile_pool(name="gth", bufs=3) as pgt,
                  tc.tile_pool(name="psg", bufs=3, space="PSUM") as ppg):
                for le in range(epc):
                    for ci in range(ct_n):
                        row = pgt.tile([128, ha], bf16, tag="row")
                        nc.sync.dma_start(
                            out=row[:],
                            in_=xg[le][ci * 128:(ci + 1) * 128, :])
                        cx = le * ct_n + ci
                        nc.vector.tensor_copy(out=wtc[:, cx:cx + 1],
                                              in_=row[:, h + le:h + le + 1])
                        tf = pgt.tile([128, 1], f32, tag="tf")
                        nc.vector.tensor_scalar(
                            tf[:], row[:, h + 2:h + 3], 32.0, scalar2=None,
                            op0=Alu.mult)
                        tf2 = pgt.tile([128, 1], f32, tag="tf2")
                        nc.vector.tensor_add(tf2[:], tf[:],
                                             row[:, h + 3:h + 4])
                        # pad slots (weight == 0) -> OOB token id so the
                        # output scatter skips them entirely
                        pb = pgt.tile([128, 1], f32, tag="pb")
                        nc.vector.tensor_scalar(
                            pb[:], wtc[:, cx:cx + 1], 0.0, scalar2=BIG,
                            op0=Alu.is_le, op1=Alu.mult)
                        tf3 = pgt.tile([128, 1], f32, tag="tf3")
                        nc.vector.tensor_add(tf3[:], tf2[:], pb[:])
                        nc.vector.tensor_copy(out=tokc[:, cx:cx + 1],
                                              in_=tf3[:])
                        for k in range(kh):
                            ptp = ppg.tile([128, 128], bf16, tag="ptp")
                            nc.tensor.transpose(
                                ptp[:, :], row[:, k * 128:(k + 1) * 128],
                                idb[:])
                            nc.vector.tensor_copy(
                                out=xgt[:, (le * kh + k) * cap + ci * 128:
                                        (le * kh + k) * cap + ci * 128 + 128],
                                in_=ptp[:, :])

            # ---- phase A: actg[f, slot] = silu(g) * u ----
            with (tc.tile_pool(name="wv", bufs=24) as pwv,
                  tc.tile_pool(name="sg", bufs=3) as psg,
                  tc.tile_pool(name="psa", bufs=4, space="PSUM") as ppa):
                for le in range(epc):
                    for m in range(kf):
                        pg = ppa.tile([128, cap], f32, tag="pg")
                        pu = ppa.tile([128, cap], f32, tag="pu")
                        for k in range(kh):
                            wg = pwv.tile([128, 128], bf16, tag="wg")
                            nc.sync.dma_start(out=wg[:], in_=wv1t[le, k, m])
                            wu = pwv.tile([128, 128], bf16, tag="wu")
                            nc.sync.dma_start(out=wu[:],
                                              in_=wv1t[le, k, m + kf])
                            rh = xgt[:, (le * kh + k) * cap:
                                     (le * kh + k) * cap + cap]
                            nc.tensor.matmul(pg[:, :], lhsT=wg[:], rhs=rh,
                                             start=(k == 0),
                                             stop=(k == kh - 1))
                            nc.tensor.matmul(pu[:, :], lhsT=wu[:], rhs=rh,
                                             start=(k == 0),
                                             stop=(k == kh - 1))
                        sgm = psg.tile([128, cap], bf16, tag="sgm")
                        nc.scalar.activation(sgm[:], pg[:], AF.Sigmoid)
                        sg = psg.tile([128, cap], bf16, tag="sg")
                        nc.vector.tensor_mul(out=sg[:], in0=sgm[:], in1=pg[:])
                        ai = (le * kf + m) * cap
                        nc.vector.tensor_mul(out=actg[:, ai:ai + cap],
                                             in0=sg[:], in1=pu[:])

            # ---- phase B: y = actg @ w2T, scale, scatter-add ----
            with (tc.tile_pool(name="w2p", bufs=kf + 3) as pw2,
                  tc.tile_pool(name="sc", bufs=2) as psc,
                  tc.tile_pool(name="psb", bufs=2, space="PSUM") as ppb):
                for le in range(epc):
                    w2ks = []
                    for k in range(kf):
                        w2k = pw2.tile([128, h], bf16, tag="w2k")
                        nc.sync.dma_start(out=w2k[:], in_=w2t[le, k])
                        w2ks.append(w2k)
                    for ci in range(ct_n):
                        cx = le * ct_n + ci
                        py = ppb.tile([128, h], f32, tag="py")
                        for k in range(kf):
                            ai = (le * kf + k) * cap + ci * 128
                            for (n0, nsz) in nh:
                                nc.tensor.matmul(
                                    py[:, n0:n0 + nsz],
                                    lhsT=actg[:, ai:ai + 128],
                                    rhs=w2ks[k][:, n0:n0 + nsz],
                                    start=(k == 0), stop=(k == kf - 1))
                        sc = psc.tile([128, h], f32, tag="sc")
                        nc.vector.tensor_scalar_mul(sc[:], py[:, :],
                                                    wtc[:, cx:cx + 1])
                        nc.gpsimd.indirect_dma_start(
                            out=partial[:, :],
                            out_offset=IndirectOffsetOnAxis(
                                ap=tokc[:, cx:cx + 1], axis=0),
                            in_=sc[:],
                            in_offset=None,
                            bounds_check=t - 1,
                            oob_is_err=False,
                            compute_op=Alu.add)

            # all scatter-adds must land before the collective reads partial
            tc.strict_bb_all_engine_barrier()

            # ---- cross-core reduce-scatter + shard output ----
            nc.gpsimd.collective_compute(
                "ReduceScatter", Alu.add,
                replica_groups=[list(range(n_cores))],
                ins=[partial.ap().opt()],
                outs=[rs_out.ap().opt()],
            )
            nc.sync.dma_start(out=out_sh[:, :], in_=rs_out[:, :])

    nc.compile()
    return nc


def prep_inputs(x, gate_w, wv1, w2, t, h, f, e, n_cores):
    """Host-side shard/cast/tile. Returns per-core input maps."""
    import ml_dtypes
    bf16 = ml_dtypes.bfloat16

    epc = e // n_cores
    kh = h // 128
    kf = f // 128
    mf2 = 2 * f // 128

    xT = np.ascontiguousarray(x.T).astype(np.float32)        # [h, t]
    xr = np.ascontiguousarray(x).astype(bf16)                # [t, h]
    toks = np.arange(t)
    thl = np.stack([toks // 32, toks % 32], axis=1).astype(bf16)
    ident = np.eye(128, dtype=np.float32)
    ones = np.ones((128, 128), dtype=np.float32)
    # sut[q, p] = 1 if q < p  (strict upper)  -> (SU.T @ m)[p] = sum_{q<p} m[q]
    sut = np.triu(np.ones((128, 128), dtype=np.float32), k=1)

    in_maps = []
    for c in range(n_cores):
        own = list(range(c * epc, (c + 1) * epc))
        rest = [i for i in range(e) if i not in own]
        perm = own + rest
        gwT = np.ascontiguousarray(gate_w[perm].T).astype(np.float32)

        wl = wv1[own]                                        # [epc, 2f, h]
        wv1tc = np.ascontiguousarray(
            wl.transpose(0, 2, 1)
              .reshape(epc, kh, 128, mf2, 128)
              .transpose(0, 1, 3, 2, 4)).astype(bf16)

        w2l = w2[own]                                        # [epc, h, f]
        w2tc = np.ascontiguousarray(
            w2l.transpose(0, 2, 1)
               .reshape(epc, kf, 128, h)).astype(bf16)

        in_maps.append({
            "xT": xT, "xr": xr, "thl": thl, "gwT": gwT,
            "wv1t": wv1tc, "w2t": w2tc,
            "ident": ident, "ones": ones, "sut": sut,
        })
    return in_maps


def kernel(x, gate_w, wv1, w2, top_k):
    from concourse.bass_utils import run_bass_kernel_spmd

    assert int(top_k) == TOPK
    x = np.asarray(x, dtype=np.float32)
    gate_w = np.asarray(gate_w, dtype=np.float32)
    wv1 = np.asarray(wv1, dtype=np.float32)
    w2 = np.asarray(w2, dtype=np.float32)

    key = (T, H, F, E, NCORES)
    if key not in _CACHE:
        _CACHE[key] = build_moe_nc(T, H, F, E, NCORES, TOPK, CAP)
    nc = _CACHE[key]

    in_maps = prep_inputs(x, gate_w, wv1, w2, T, H, F, E, NCORES)
    res = run_bass_kernel_spmd(nc, in_maps, list(range(NCORES)))
    shards = [res.results[c]["out_shard"] for c in range(NCORES)]
    return np.concatenate(shards, axis=0).astype(np.float32)


# revision 21
# speedup vs baseline: 1.1456x; 1.1432x over previous
"""Block-sparse MoE (dense expert-parallel) Trainium2 kernel.

Problem: nn_BlockSparseMoE_15882789061249
  T=1024 tokens, H=2048 hidden, F=1408 intermediate, E=16 experts, top_k=6.

Strategy (8 NeuronCores, SPMD single program):
  - Expert parallel: core c owns experts {2c, 2c+1}. wv1/w2 sharded by
    expert on the host; x and the gate are replicated (x is 8 MB vs 554 MB
    of weights, so replicating x beats an all-to-all token dispatch at this
    scale).
  - Host permutes the gate columns per core so that each core's own two
    experts land in route columns 0 and 1 -> a single SPMD program works
    for every core (top-k mask / renormalization are permutation-invariant).
  - On-core: fp32 router (logits -> exp -> top-6 via DVE max8/match_replace
    -> renormalized weights), bf16 expert matmuls (weights pre-cast and
    pre-tiled on host), SiLU on ScalarE, per-token combine via per-partition
    scalar multiply, DMA-accumulate of the two local experts into a DRAM
    partial, then an 8-core ReduceScatter; each core emits its 128-token
    output shard and the host concatenates shards.
"""

import numpy as np

T, H, F, E = 1024, 2048, 1408, 16
NCORES = 8
TOPK = 6

_CACHE = {}


def build_moe_nc(t, h, f, e, n_cores, topk=6):
    """Build + compile the SPMD Bass program for one core (same for all)."""
    import concourse.bacc as bacc
    import concourse.mybir as mybir
    import concourse.tile as tile

    f32 = mybir.dt.float32
    bf16 = mybir.dt.bfloat16
    AF = mybir.ActivationFunctionType
    Alu = mybir.AluOpType
    X = mybir.AxisListType.X

    epc = e // n_cores          # experts per core
    kh = h // 128               # contraction tiles over hidden
    kf = f // 128               # contraction tiles over intermediate
    mt = t // 128               # token tiles
    mf2 = 2 * f // 128          # fused gate+up row tiles
    tsh = t // n_cores          # output shard tokens
    nt = [(i, min(512, t - i)) for i in range(0, t, 512)]
    nh = [(i, min(512, h - i)) for i in range(0, h, 512)]

    nc = bacc.Bacc("TRN2", target_bir_lowering=False, debug=False,
                   num_devices=n_cores)

    xT = nc.dram_tensor("xT", [h, t], f32, kind="ExternalInput")
    gwT = nc.dram_tensor("gwT", [h, e], f32, kind="ExternalInput")
    wv1t = nc.dram_tensor("wv1t", [epc, kh, mf2, 128, 128], bf16,
                          kind="ExternalInput")
    w2t = nc.dram_tensor("w2t", [epc, kf, 128, h], bf16, kind="ExternalInput")
    ident = nc.dram_tensor("ident", [128, 128], f32, kind="ExternalInput")
    out_sh = nc.dram_tensor("out_shard", [tsh, h], f32, kind="ExternalOutput")

    partial = nc.dram_tensor("partial", [t, h], f32)
    ch = t // 2                      # reduce-scatter chunk (token rows)
    chs = ch // n_cores              # per-rank rows per chunk
    rs0 = nc.dram_tensor("rs0", [chs, h], f32)
    rs1 = nc.dram_tensor("rs1", [chs, h], f32)

    with tile.TileContext(nc) as tc:
        with tc.tile_pool(name="persist", bufs=1) as pp:
            xb = pp.tile([128, kh * t], bf16, tag="xb")
            gw = pp.tile([128, kh * e], f32, tag="gw")
            ids = pp.tile([128, 128], f32, tag="ids")
            route = pp.tile([128, mt * e], f32, tag="route")
            act = pp.tile([128, epc * kf * t], bf16, tag="act")
            lg = pp.tile([128, t], f32, tag="lg")

            nc.sync.dma_start(out=ids[:], in_=ident[:, :])
            for k in range(kh):
                nc.sync.dma_start(out=gw[:, k * e:(k + 1) * e],
                                  in_=gwT[k * 128:(k + 1) * 128, :])

            # ---- load x (fp32), cast to bf16, router logits [e, t] ----
            with (tc.tile_pool(name="xload", bufs=3) as pxl,
                  tc.tile_pool(name="psr", bufs=1, space="PSUM") as ppr):
                psl = ppr.tile([128, t], f32, tag="psl")
                for k in range(kh):
                    xf = pxl.tile([128, t], f32, tag="xf")
                    nc.sync.dma_start(out=xf[:],
                                      in_=xT[k * 128:(k + 1) * 128, :])
                    nc.vector.tensor_copy(out=xb[:, k * t:(k + 1) * t],
                                          in_=xf[:])
                    for (n0, nsz) in nt:
                        nc.tensor.matmul(
                            psl[:e, n0:n0 + nsz],
                            lhsT=gw[:, k * e:(k + 1) * e],
                            rhs=xf[:, n0:n0 + nsz],
                            start=(k == 0), stop=(k == kh - 1))
                nc.vector.tensor_copy(out=lg[:e, :], in_=psl[:e, :])

            # ---- router: per token tile, top-k renormalized weights ----
            with (tc.tile_pool(name="rt", bufs=2) as prt,
                  tc.tile_pool(name="pst", bufs=2, space="PSUM") as ppt):
                for tt in range(mt):
                    ptile = ppt.tile([128, e], f32, tag="ltr")
                    nc.tensor.transpose(ptile[:, :],
                                        lg[:e, tt * 128:(tt + 1) * 128],
                                        ids[:e, :e])
                    mx = prt.tile([128, 1], f32, tag="mx")
                    nc.vector.reduce_max(out=mx[:], in_=ptile[:, :], axis=X)
                    nm = prt.tile([128, 1], f32, tag="nm")
                    nc.vector.tensor_scalar_mul(nm[:], mx[:], -1.0)
                    ev = prt.tile([128, e], f32, tag="ev")
                    nc.scalar.activation(ev[:], ptile[:, :], AF.Exp,
                                         bias=nm[:], scale=1.0)
                    t8 = prt.tile([128, 8], f32, tag="t8")
                    nc.vector.max(out=t8[:], in_=ev[:])
                    if topk < 8:
                        nc.vector.memset(t8[:, topk:], 0.0)
                    zap = prt.tile([128, e], f32, tag="zap")
                    nc.vector.match_replace(out=zap[:], in_to_replace=t8[:],
                                            in_values=ev[:], imm_value=0.0)
                    msk = prt.tile([128, e], f32, tag="msk")
                    nc.vector.tensor_sub(msk[:], ev[:], zap[:])
                    dn = prt.tile([128, 1], f32, tag="dn")
                    nc.vector.reduce_sum(out=dn[:], in_=msk[:], axis=X)
                    iv = prt.tile([128, 1], f32, tag="iv")
                    nc.vector.reciprocal(iv[:], dn[:])
                    nc.vector.tensor_scalar_mul(
                        route[:, tt * e:(tt + 1) * e], msk[:], iv[:])

            # ---- phase A: act[f, t] = silu(g) * u per local expert ----
            with (tc.tile_pool(name="wv", bufs=4) as pwv,
                  tc.tile_pool(name="sg", bufs=2) as psg,
                  tc.tile_pool(name="psa", bufs=2, space="PSUM") as ppa):
                for le in range(epc):
                    for m in range(kf):
                        pg = ppa.tile([128, t], f32, tag="pg")
                        pu = ppa.tile([128, t], f32, tag="pu")
                        for k in range(kh):
                            wg = pwv.tile([128, 128], bf16, tag="wg")
                            nc.sync.dma_start(out=wg[:], in_=wv1t[le, k, m])
                            wu = pwv.tile([128, 128], bf16, tag="wu")
                            nc.sync.dma_start(out=wu[:],
                                              in_=wv1t[le, k, m + kf])
                            # one weight load serves both N-halves
                            for (n0, nsz) in nt:
                                rh = xb[:, k * t + n0:k * t + n0 + nsz]
                                nc.tensor.matmul(pg[:, n0:n0 + nsz],
                                                 lhsT=wg[:], rhs=rh,
                                                 start=(k == 0),
                                                 stop=(k == kh - 1))
                            for (n0, nsz) in nt:
                                rh = xb[:, k * t + n0:k * t + n0 + nsz]
                                nc.tensor.matmul(pu[:, n0:n0 + nsz],
                                                 lhsT=wu[:], rhs=rh,
                                                 start=(k == 0),
                                                 stop=(k == kh - 1))
                        sgm = psg.tile([128, t], bf16, tag="sgm")
                        nc.scalar.activation(sgm[:], pg[:], AF.Sigmoid)
                        sg = psg.tile([128, t], bf16, tag="sg")
                        nc.vector.tensor_mul(out=sg[:], in0=sgm[:], in1=pg[:])
                        ai = (le * kf + m) * t
                        nc.vector.tensor_mul(out=act[:, ai:ai + t],
                                             in0=sg[:], in1=pu[:])

            # ---- phase B: y = act @ w2T, combine with route weights ----
            with (tc.tile_pool(name="w2p", bufs=kf + 3) as pw2,
                  tc.tile_pool(name="sc", bufs=3) as psc,
                  tc.tile_pool(name="psb", bufs=2, space="PSUM") as ppb):
                for le in range(epc):
                    w2ks = []
                    for k in range(kf):
                        w2k = pw2.tile([128, h], bf16, tag="w2k")
                        nc.sync.dma_start(out=w2k[:], in_=w2t[le, k])
                        w2ks.append(w2k)
                    for tt in range(mt):
                        py = ppb.tile([128, h], f32, tag="py")
                        for k in range(kf):
                            ai = (le * kf + k) * t + tt * 128
                            for (n0, nsz) in nh:
                                nc.tensor.matmul(
                                    py[:, n0:n0 + nsz],
                                    lhsT=act[:, ai:ai + 128],
                                    rhs=w2ks[k][:, n0:n0 + nsz],
                                    start=(k == 0), stop=(k == kf - 1))
                        rcol = route[:, tt * e + le:tt * e + le + 1]
                        sc = psc.tile([128, h], f32, tag="sc")
                        nc.vector.tensor_scalar_mul(sc[:], py[:, :], rcol)
                        dst = partial[tt * 128:(tt + 1) * 128, :]
                        if le == 0:
                            nc.sync.dma_start(out=dst, in_=sc[:])
                        else:
                            nc.gpsimd.dma_start(out=dst, in_=sc[:],
                                                accum_op=Alu.add)
                        # first-half tokens are final after the last
                        # expert's mid-point: reduce-scatter them while the
                        # second half still computes
                        if le == epc - 1 and tt == mt // 2 - 1:
                            nc.gpsimd.collective_compute(
                                "ReduceScatter", Alu.add,
                                replica_groups=[list(range(n_cores))],
                                ins=[partial[0:ch, :].opt()],
                                outs=[rs0.ap().opt()],
                            )
                            nc.sync.dma_start(out=out_sh[0:chs, :],
                                              in_=rs0[:, :])

            # ---- second-half reduce-scatter + shard output ----
            nc.gpsimd.collective_compute(
                "ReduceScatter", Alu.add,
                replica_groups=[list(range(n_cores))],
                ins=[partial[ch:t, :].opt()],
                outs=[rs1.ap().opt()],
            )
            nc.sync.dma_start(out=out_sh[chs:tsh, :], in_=rs1[:, :])

    nc.compile()
    return nc


def prep_inputs(x, gate_w, wv1, w2, t, h, f, e, n_cores):
    """Host-side shard/cast/tile. Returns per-core input maps."""
    import ml_dtypes
    bf16 = ml_dtypes.bfloat16

    epc = e // n_cores
    kh = h // 128
    kf = f // 128
    mf2 = 2 * f // 128

    xT = np.ascontiguousarray(x.T).astype(np.float32)        # [h, t]
    ident = np.eye(128, dtype=np.float32)

    in_maps = []
    for c in range(n_cores):
        own = list(range(c * epc, (c + 1) * epc))
        rest = [i for i in range(e) if i not in own]
        perm = own + rest
        gwT = np.ascontiguousarray(gate_w[perm].T).astype(np.float32)

        wl = wv1[own]                                        # [epc, 2f, h]
        # wv1t[le, k, m, hp, fp] = wv1[own[le], m*128+fp, k*128+hp]
        wv1tc = np.ascontiguousarray(
            wl.transpose(0, 2, 1)                            # [epc, h, 2f]
              .reshape(epc, kh, 128, mf2, 128)
              .transpose(0, 1, 3, 2, 4)).astype(bf16)

        w2l = w2[own]                                        # [epc, h, f]
        # w2t[le, k, fp, hh] = w2[own[le], hh, k*128+fp]
        w2tc = np.ascontiguousarray(
            w2l.transpose(0, 2, 1)                           # [epc, f, h]
               .reshape(epc, kf, 128, h)).astype(bf16)

        in_maps.append({
            "xT": xT,
            "gwT": gwT,
            "wv1t": wv1tc,
            "w2t": w2tc,
            "ident": ident,
        })
    return in_maps


def unshard(shards, t, h, n_cores):
    """Reassemble the full output from per-core chunked-RS shards.

    Core c's shard rows [0:chs] are tokens [c*chs:(c+1)*chs] (chunk 0) and
    rows [chs:2*chs] are tokens [t//2 + c*chs : t//2 + (c+1)*chs] (chunk 1).
    """
    ch = t // 2
    chs = ch // n_cores
    out = np.empty((t, h), dtype=np.float32)
    for c, sh in enumerate(shards):
        out[c * chs:(c + 1) * chs] = sh[:chs]
        out[ch + c * chs:ch + (c + 1) * chs] = sh[chs:]
    return out


def kernel(x, gate_w, wv1, w2, top_k):
    from concourse.bass_utils import run_bass_kernel_spmd

    assert int(top_k) == TOPK
    x = np.asarray(x, dtype=np.float32)
    gate_w = np.asarray(gate_w, dtype=np.float32)
    wv1 = np.asarray(wv1, dtype=np.float32)
    w2 = np.asarray(w2, dtype=np.float32)

    key = (T, H, F, E, NCORES)
    if key not in _CACHE:
        _CACHE[key] = build_moe_nc(T, H, F, E, NCORES, TOPK)
    nc = _CACHE[key]

    in_maps = prep_inputs(x, gate_w, wv1, w2, T, H, F, E, NCORES)
    res = run_bass_kernel_spmd(nc, in_maps, list(range(NCORES)))
    shards = [res.results[c]["out_shard"] for c in range(NCORES)]
    return unshard(shards, T, H, NCORES)


# revision 26
# speedup vs baseline: 1.2256x; 1.0698x over previous
"""Block-sparse MoE (dense expert-parallel) Trainium2 kernel.

Problem: nn_BlockSparseMoE_15882789061249
  T=1024 tokens, H=2048 hidden, F=1408 intermediate, E=16 experts, top_k=6.

Strategy (8 NeuronCores, SPMD single program):
  - Expert parallel: core c owns experts {2c, 2c+1}. wv1/w2 sharded by
    expert on the host; x and the gate are replicated (x is 8 MB vs 554 MB
    of weights, so replicating x beats an all-to-all token dispatch at this
    scale).
  - Host permutes the gate columns per core so that each core's own two
    experts land in route columns 0 and 1 -> a single SPMD program works
    for every core (top-k mask / renormalization are permutation-invariant).
  - On-core: fp32 router (logits -> exp -> top-6 via DVE max8/match_replace
    -> renormalized weights), bf16 expert matmuls (weights pre-cast and
    pre-tiled on host), SiLU on ScalarE, per-token combine via per-partition
    scalar multiply, DMA-accumulate of the two local experts into a DRAM
    partial, then an 8-core ReduceScatter; each core emits its 128-token
    output shard and the host concatenates shards.
"""

import numpy as np

T, H, F, E = 1024, 2048, 1408, 16
NCORES = 8
TOPK = 6

_CACHE = {}


def build_moe_nc(t, h, f, e, n_cores, topk=6):
    """Build + compile the SPMD Bass program for one core (same for all)."""
    import concourse.bacc as bacc
    import concourse.mybir as mybir
    import concourse.tile as tile

    f32 = mybir.dt.float32
    bf16 = mybir.dt.bfloat16
    AF = mybir.ActivationFunctionType
    Alu = mybir.AluOpType
    X = mybir.AxisListType.X

    epc = e // n_cores          # experts per core
    kh = h // 128               # contraction tiles over hidden
    kf = f // 128               # contraction tiles over intermediate
    mt = t // 128               # token tiles
    mf2 = 2 * f // 128          # fused gate+up row tiles
    tsh = t // n_cores          # output shard tokens
    nt = [(i, min(512, t - i)) for i in range(0, t, 512)]
    nh = [(i, min(512, h - i)) for i in range(0, h, 512)]

    nc = bacc.Bacc("TRN2", target_bir_lowering=False, debug=False,
                   num_devices=n_cores)

    xT = nc.dram_tensor("xT", [h, t], f32, kind="ExternalInput")
    gwT = nc.dram_tensor("gwT", [h, e], f32, kind="ExternalInput")
    wv1t = nc.dram_tensor("wv1t", [epc, kh, mf2, 128, 128], bf16,
                          kind="ExternalInput")
    w2t = nc.dram_tensor("w2t", [epc, kf, 128, h], bf16, kind="ExternalInput")
    ident = nc.dram_tensor("ident", [128, 128], f32, kind="ExternalInput")
    out_sh = nc.dram_tensor("out_shard", [tsh, h], f32, kind="ExternalOutput")

    # partial + collective run in bf16: halves accumulate-DMA and
    # reduce-scatter traffic; adds ~0.3% absmax error (budget is 2e-2)
    partial = nc.dram_tensor("partial", [t, h], bf16)
    rs_out = nc.dram_tensor("rs_out", [tsh, h], bf16)

    with tile.TileContext(nc) as tc:
        with tc.tile_pool(name="persist", bufs=1) as pp:
            xb = pp.tile([128, kh * t], bf16, tag="xb")
            gw = pp.tile([128, kh * e], f32, tag="gw")
            ids = pp.tile([128, 128], f32, tag="ids")
            route = pp.tile([128, mt * e], f32, tag="route")
            act = pp.tile([128, epc * kf * t], bf16, tag="act")
            lg = pp.tile([128, t], f32, tag="lg")

            nc.sync.dma_start(out=ids[:], in_=ident[:, :])
            for k in range(kh):
                nc.sync.dma_start(out=gw[:, k * e:(k + 1) * e],
                                  in_=gwT[k * 128:(k + 1) * 128, :])

            # ---- load x (fp32), cast to bf16, router logits [e, t] ----
            with (tc.tile_pool(name="xload", bufs=3) as pxl,
                  tc.tile_pool(name="psr", bufs=1, space="PSUM") as ppr):
                psl = ppr.tile([128, t], f32, tag="psl")
                for k in range(kh):
                    xf = pxl.tile([128, t], f32, tag="xf")
                    nc.sync.dma_start(out=xf[:],
                                      in_=xT[k * 128:(k + 1) * 128, :])
                    nc.vector.tensor_copy(out=xb[:, k * t:(k + 1) * t],
                                          in_=xf[:])
                    for (n0, nsz) in nt:
                        nc.tensor.matmul(
                            psl[:e, n0:n0 + nsz],
                            lhsT=gw[:, k * e:(k + 1) * e],
                            rhs=xf[:, n0:n0 + nsz],
                            start=(k == 0), stop=(k == kh - 1))
                nc.vector.tensor_copy(out=lg[:e, :], in_=psl[:e, :])

            # ---- router: per token tile, top-k renormalized weights ----
            with (tc.tile_pool(name="rt", bufs=2) as prt,
                  tc.tile_pool(name="pst", bufs=2, space="PSUM") as ppt):
                for tt in range(mt):
                    ptile = ppt.tile([128, e], f32, tag="ltr")
                    nc.tensor.transpose(ptile[:, :],
                                        lg[:e, tt * 128:(tt + 1) * 128],
                                        ids[:e, :e])
                    mx = prt.tile([128, 1], f32, tag="mx")
                    nc.vector.reduce_max(out=mx[:], in_=ptile[:, :], axis=X)
                    nm = prt.tile([128, 1], f32, tag="nm")
                    nc.vector.tensor_scalar_mul(nm[:], mx[:], -1.0)
                    ev = prt.tile([128, e], f32, tag="ev")
                    nc.scalar.activation(ev[:], ptile[:, :], AF.Exp,
                                         bias=nm[:], scale=1.0)
                    t8 = prt.tile([128, 8], f32, tag="t8")
                    nc.vector.max(out=t8[:], in_=ev[:])
                    if topk < 8:
                        nc.vector.memset(t8[:, topk:], 0.0)
                    zap = prt.tile([128, e], f32, tag="zap")
                    nc.vector.match_replace(out=zap[:], in_to_replace=t8[:],
                                            in_values=ev[:], imm_value=0.0)
                    msk = prt.tile([128, e], f32, tag="msk")
                    nc.vector.tensor_sub(msk[:], ev[:], zap[:])
                    dn = prt.tile([128, 1], f32, tag="dn")
                    nc.vector.reduce_sum(out=dn[:], in_=msk[:], axis=X)
                    iv = prt.tile([128, 1], f32, tag="iv")
                    nc.vector.reciprocal(iv[:], dn[:])
                    nc.vector.tensor_scalar_mul(
                        route[:, tt * e:(tt + 1) * e], msk[:], iv[:])

            # ---- phase A: act[f, t] = silu(g) * u per local expert ----
            with (tc.tile_pool(name="wv", bufs=16) as pwv,
                  tc.tile_pool(name="sg", bufs=3) as psg,
                  tc.tile_pool(name="psa", bufs=2, space="PSUM") as ppa):
                for le in range(epc):
                    for m in range(kf):
                        pg = ppa.tile([128, t], f32, tag="pg")
                        pu = ppa.tile([128, t], f32, tag="pu")
                        for k in range(kh):
                            wg = pwv.tile([128, 128], bf16, tag="wg")
                            nc.sync.dma_start(out=wg[:], in_=wv1t[le, k, m])
                            wu = pwv.tile([128, 128], bf16, tag="wu")
                            nc.sync.dma_start(out=wu[:],
                                              in_=wv1t[le, k, m + kf])
                            # one weight load serves both N-halves
                            for (n0, nsz) in nt:
                                rh = xb[:, k * t + n0:k * t + n0 + nsz]
                                nc.tensor.matmul(pg[:, n0:n0 + nsz],
                                                 lhsT=wg[:], rhs=rh,
                                                 start=(k == 0),
                                                 stop=(k == kh - 1))
                            for (n0, nsz) in nt:
                                rh = xb[:, k * t + n0:k * t + n0 + nsz]
                                nc.tensor.matmul(pu[:, n0:n0 + nsz],
                                                 lhsT=wu[:], rhs=rh,
                                                 start=(k == 0),
                                                 stop=(k == kh - 1))
                        sgm = psg.tile([128, t], bf16, tag="sgm")
                        nc.scalar.activation(sgm[:], pg[:], AF.Sigmoid)
                        sg = psg.tile([128, t], bf16, tag="sg")
                        nc.vector.tensor_mul(out=sg[:], in0=sgm[:], in1=pg[:])
                        ai = (le * kf + m) * t
                        nc.vector.tensor_mul(out=act[:, ai:ai + t],
                                             in0=sg[:], in1=pu[:])

            # ---- phase B: y = act @ w2T, combine with route weights ----
            with (tc.tile_pool(name="w2p", bufs=kf + 3) as pw2,
                  tc.tile_pool(name="sc", bufs=3) as psc,
                  tc.tile_pool(name="psb", bufs=2, space="PSUM") as ppb):
                for le in range(epc):
                    w2ks = []
                    for k in range(kf):
                        w2k = pw2.tile([128, h], bf16, tag="w2k")
                        nc.sync.dma_start(out=w2k[:], in_=w2t[le, k])
                        w2ks.append(w2k)
                    for tt in range(mt):
                        py = ppb.tile([128, h], f32, tag="py")
                        for k in range(kf):
                            ai = (le * kf + k) * t + tt * 128
                            for (n0, nsz) in nh:
                                nc.tensor.matmul(
                                    py[:, n0:n0 + nsz],
                                    lhsT=act[:, ai:ai + 128],
                                    rhs=w2ks[k][:, n0:n0 + nsz],
                                    start=(k == 0), stop=(k == kf - 1))
                        rcol = route[:, tt * e + le:tt * e + le + 1]
                        sc = psc.tile([128, h], bf16, tag="sc")
                        nc.vector.tensor_scalar_mul(sc[:], py[:, :], rcol)
                        dst = partial[tt * 128:(tt + 1) * 128, :]
                        if le == 0:
                            nc.sync.dma_start(out=dst, in_=sc[:])
                        else:
                            nc.gpsimd.dma_start(out=dst, in_=sc[:],
                                                accum_op=Alu.add)

            # ---- cross-core reduce-scatter + shard output (fp32 out) ----
            nc.gpsimd.collective_compute(
                "ReduceScatter", Alu.add,
                replica_groups=[list(range(n_cores))],
                ins=[partial.ap().opt()],
                outs=[rs_out.ap().opt()],
            )
            with tc.tile_pool(name="cvt", bufs=2) as pcv:
                rw = min(128, tsh)
                for rb in range(0, tsh, rw):
                    rt_ = pcv.tile([rw, h], bf16, tag="rt_")
                    nc.sync.dma_start(out=rt_[:], in_=rs_out[rb:rb + rw, :])
                    cv = pcv.tile([rw, h], f32, tag="cv")
                    nc.vector.tensor_copy(out=cv[:], in_=rt_[:])
                    nc.sync.dma_start(out=out_sh[rb:rb + rw, :], in_=cv[:])

    nc.compile()
    return nc


def prep_inputs(x, gate_w, wv1, w2, t, h, f, e, n_cores):
    """Host-side shard/cast/tile. Returns per-core input maps."""
    import ml_dtypes
    bf16 = ml_dtypes.bfloat16

    epc = e // n_cores
    kh = h // 128
    kf = f // 128
    mf2 = 2 * f // 128

    xT = np.ascontiguousarray(x.T).astype(np.float32)        # [h, t]
    ident = np.eye(128, dtype=np.float32)

    in_maps = []
    for c in range(n_cores):
        own = list(range(c * epc, (c + 1) * epc))
        rest = [i for i in range(e) if i not in own]
        perm = own + rest
        gwT = np.ascontiguousarray(gate_w[perm].T).astype(np.float32)

        wl = wv1[own]                                        # [epc, 2f, h]
        # wv1t[le, k, m, hp, fp] = wv1[own[le], m*128+fp, k*128+hp]
        wv1tc = np.ascontiguousarray(
            wl.transpose(0, 2, 1)                            # [epc, h, 2f]
              .reshape(epc, kh, 128, mf2, 128)
              .transpose(0, 1, 3, 2, 4)).astype(bf16)

        w2l = w2[own]                                        # [epc, h, f]
        # w2t[le, k, fp, hh] = w2[own[le], hh, k*128+fp]
        w2tc = np.ascontiguousarray(
            w2l.transpose(0, 2, 1)                           # [epc, f, h]
               .reshape(epc, kf, 128, h)).astype(bf16)

        in_maps.append({
            "xT": xT,
            "gwT": gwT,
            "wv1t": wv1tc,
            "w2t": w2tc,
            "ident": ident,
        })
    return in_maps


def unshard(shards, t, h, n_cores):
    """Reassemble the full output from per-core RS shards (rank order)."""
    return np.concatenate(shards, axis=0).astype(np.float32)


def kernel(x, gate_w, wv1, w2, top_k):
    from concourse.bass_utils import run_bass_kernel_spmd

    assert int(top_k) == TOPK
    x = np.asarray(x, dtype=np.float32)
    gate_w = np.asarray(gate_w, dtype=np.float32)
    wv1 = np.asarray(wv1, dtype=np.float32)
    w2 = np.asarray(w2, dtype=np.float32)

    key = (T, H, F, E, NCORES)
    if key not in _CACHE:
        _CACHE[key] = build_moe_nc(T, H, F, E, NCORES, TOPK)
    nc = _CACHE[key]

    in_maps = prep_inputs(x, gate_w, wv1, w2, T, H, F, E, NCORES)
    res = run_bass_kernel_spmd(nc, in_maps, list(range(NCORES)))
    shards = [res.results[c]["out_shard"] for c in range(NCORES)]
    return unshard(shards, T, H, NCORES)


# revision 27
# speedup vs baseline: 1.2329x; 1.0059x over previous
"""Block-sparse MoE (dense expert-parallel) Trainium2 kernel.

Problem: nn_BlockSparseMoE_15882789061249
  T=1024 tokens, H=2048 hidden, F=1408 intermediate, E=16 experts, top_k=6.

Strategy (8 NeuronCores, SPMD single program):
  - Expert parallel: core c owns experts {2c, 2c+1}. wv1/w2 sharded by
    expert on the host; x and the gate are replicated (x is 8 MB vs 554 MB
    of weights, so replicating x beats an all-to-all token dispatch at this
    scale).
  - Host permutes the gate columns per core so that each core's own two
    experts land in route columns 0 and 1 -> a single SPMD program works
    for every core (top-k mask / renormalization are permutation-invariant).
  - On-core: fp32 router (logits -> exp -> top-6 via DVE max8/match_replace
    -> renormalized weights), bf16 expert matmuls (weights pre-cast and
    pre-tiled on host), SiLU on ScalarE, per-token combine via per-partition
    scalar multiply, DMA-accumulate of the two local experts into a DRAM
    partial, then an 8-core ReduceScatter; each core emits its 128-token
    output shard and the host concatenates shards.
"""

import numpy as np

T, H, F, E = 1024, 2048, 1408, 16
NCORES = 8
TOPK = 6

_CACHE = {}


def build_moe_nc(t, h, f, e, n_cores, topk=6):
    """Build + compile the SPMD Bass program for one core (same for all)."""
    import concourse.bacc as bacc
    import concourse.mybir as mybir
    import concourse.tile as tile

    f32 = mybir.dt.float32
    bf16 = mybir.dt.bfloat16
    AF = mybir.ActivationFunctionType
    Alu = mybir.AluOpType
    X = mybir.AxisListType.X

    epc = e // n_cores          # experts per core
    kh = h // 128               # contraction tiles over hidden
    kf = f // 128               # contraction tiles over intermediate
    mt = t // 128               # token tiles
    mf2 = 2 * f // 128          # fused gate+up row tiles
    tsh = t // n_cores          # output shard tokens
    nt = [(i, min(512, t - i)) for i in range(0, t, 512)]
    nh = [(i, min(512, h - i)) for i in range(0, h, 512)]

    nc = bacc.Bacc("TRN2", target_bir_lowering=False, debug=False,
                   num_devices=n_cores)

    xT = nc.dram_tensor("xT", [h, t], f32, kind="ExternalInput")
    gwT = nc.dram_tensor("gwT", [h, e], f32, kind="ExternalInput")
    wv1t = nc.dram_tensor("wv1t", [epc, kh, mf2, 128, 128], bf16,
                          kind="ExternalInput")
    w2t = nc.dram_tensor("w2t", [epc, kf, 128, h], bf16, kind="ExternalInput")
    ident = nc.dram_tensor("ident", [128, 128], f32, kind="ExternalInput")
    out_sh = nc.dram_tensor("out_shard", [tsh, h], f32, kind="ExternalOutput")

    # partial + collective run in bf16: halves accumulate-DMA and
    # reduce-scatter traffic; adds ~0.3% absmax error (budget is 2e-2)
    partial = nc.dram_tensor("partial", [t, h], bf16)
    rs_out = nc.dram_tensor("rs_out", [tsh, h], bf16)

    with tile.TileContext(nc) as tc:
        with tc.tile_pool(name="persist", bufs=1) as pp:
            xb = pp.tile([128, kh * t], bf16, tag="xb")
            gw = pp.tile([128, kh * e], f32, tag="gw")
            ids = pp.tile([128, 128], f32, tag="ids")
            route = pp.tile([128, mt * e], f32, tag="route")
            act = pp.tile([128, epc * kf * t], bf16, tag="act")
            lg = pp.tile([128, t], f32, tag="lg")

            nc.sync.dma_start(out=ids[:], in_=ident[:, :])
            for k in range(kh):
                nc.sync.dma_start(out=gw[:, k * e:(k + 1) * e],
                                  in_=gwT[k * 128:(k + 1) * 128, :])

            # ---- load x (fp32), cast to bf16, router logits [e, t] ----
            with (tc.tile_pool(name="xload", bufs=6) as pxl,
                  tc.tile_pool(name="psr", bufs=1, space="PSUM") as ppr):
                psl = ppr.tile([128, t], f32, tag="psl")
                for k in range(kh):
                    xf = pxl.tile([128, t], f32, tag="xf")
                    nc.sync.dma_start(out=xf[:],
                                      in_=xT[k * 128:(k + 1) * 128, :])
                    nc.vector.tensor_copy(out=xb[:, k * t:(k + 1) * t],
                                          in_=xf[:])
                    for (n0, nsz) in nt:
                        nc.tensor.matmul(
                            psl[:e, n0:n0 + nsz],
                            lhsT=gw[:, k * e:(k + 1) * e],
                            rhs=xf[:, n0:n0 + nsz],
                            start=(k == 0), stop=(k == kh - 1))
                nc.vector.tensor_copy(out=lg[:e, :], in_=psl[:e, :])

            # ---- router: per token tile, top-k renormalized weights ----
            with (tc.tile_pool(name="rt", bufs=2) as prt,
                  tc.tile_pool(name="pst", bufs=2, space="PSUM") as ppt):
                for tt in range(mt):
                    ptile = ppt.tile([128, e], f32, tag="ltr")
                    nc.tensor.transpose(ptile[:, :],
                                        lg[:e, tt * 128:(tt + 1) * 128],
                                        ids[:e, :e])
                    mx = prt.tile([128, 1], f32, tag="mx")
                    nc.vector.reduce_max(out=mx[:], in_=ptile[:, :], axis=X)
                    nm = prt.tile([128, 1], f32, tag="nm")
                    nc.vector.tensor_scalar_mul(nm[:], mx[:], -1.0)
                    ev = prt.tile([128, e], f32, tag="ev")
                    nc.scalar.activation(ev[:], ptile[:, :], AF.Exp,
                                         bias=nm[:], scale=1.0)
                    t8 = prt.tile([128, 8], f32, tag="t8")
                    nc.vector.max(out=t8[:], in_=ev[:])
                    if topk < 8:
                        nc.vector.memset(t8[:, topk:], 0.0)
                    zap = prt.tile([128, e], f32, tag="zap")
                    nc.vector.match_replace(out=zap[:], in_to_replace=t8[:],
                                            in_values=ev[:], imm_value=0.0)
                    msk = prt.tile([128, e], f32, tag="msk")
                    nc.vector.tensor_sub(msk[:], ev[:], zap[:])
                    dn = prt.tile([128, 1], f32, tag="dn")
                    nc.vector.reduce_sum(out=dn[:], in_=msk[:], axis=X)
                    iv = prt.tile([128, 1], f32, tag="iv")
                    nc.vector.reciprocal(iv[:], dn[:])
                    nc.vector.tensor_scalar_mul(
                        route[:, tt * e:(tt + 1) * e], msk[:], iv[:])

            # ---- phase A: act[f, t] = silu(g) * u per local expert ----
            with (tc.tile_pool(name="wv", bufs=16) as pwv,
                  tc.tile_pool(name="sg", bufs=3) as psg,
                  tc.tile_pool(name="psa", bufs=2, space="PSUM") as ppa):
                for le in range(epc):
                    for m in range(kf):
                        pg = ppa.tile([128, t], f32, tag="pg")
                        pu = ppa.tile([128, t], f32, tag="pu")
                        for k in range(kh):
                            wg = pwv.tile([128, 128], bf16, tag="wg")
                            nc.sync.dma_start(out=wg[:], in_=wv1t[le, k, m])
                            wu = pwv.tile([128, 128], bf16, tag="wu")
                            nc.sync.dma_start(out=wu[:],
                                              in_=wv1t[le, k, m + kf])
                            # one weight load serves both N-halves
                            for (n0, nsz) in nt:
                                rh = xb[:, k * t + n0:k * t + n0 + nsz]
                                nc.tensor.matmul(pg[:, n0:n0 + nsz],
                                                 lhsT=wg[:], rhs=rh,
                                                 start=(k == 0),
                                                 stop=(k == kh - 1))
                            for (n0, nsz) in nt:
                                rh = xb[:, k * t + n0:k * t + n0 + nsz]
                                nc.tensor.matmul(pu[:, n0:n0 + nsz],
                                                 lhsT=wu[:], rhs=rh,
                                                 start=(k == 0),
                                                 stop=(k == kh - 1))
                        sgm = psg.tile([128, t], bf16, tag="sgm")
                        nc.scalar.activation(sgm[:], pg[:], AF.Sigmoid)
                        sg = psg.tile([128, t], bf16, tag="sg")
                        nc.vector.tensor_mul(out=sg[:], in0=sgm[:], in1=pg[:])
                        ai = (le * kf + m) * t
                        nc.vector.tensor_mul(out=act[:, ai:ai + t],
                                             in0=sg[:], in1=pu[:])

            # ---- phase B: y = act @ w2T, combine with route weights ----
            with (tc.tile_pool(name="w2p", bufs=kf + 3) as pw2,
                  tc.tile_pool(name="sc", bufs=3) as psc,
                  tc.tile_pool(name="psb", bufs=2, space="PSUM") as ppb):
                for le in range(epc):
                    w2ks = []
                    for k in range(kf):
                        w2k = pw2.tile([128, h], bf16, tag="w2k")
                        nc.sync.dma_start(out=w2k[:], in_=w2t[le, k])
                        w2ks.append(w2k)
                    for tt in range(mt):
                        py = ppb.tile([128, h], f32, tag="py")
                        for k in range(kf):
                            ai = (le * kf + k) * t + tt * 128
                            for (n0, nsz) in nh:
                                nc.tensor.matmul(
                                    py[:, n0:n0 + nsz],
                                    lhsT=act[:, ai:ai + 128],
                                    rhs=w2ks[k][:, n0:n0 + nsz],
                                    start=(k == 0), stop=(k == kf - 1))
                        rcol = route[:, tt * e + le:tt * e + le + 1]
                        sc = psc.tile([128, h], bf16, tag="sc")
                        nc.vector.tensor_scalar_mul(sc[:], py[:, :], rcol)
                        dst = partial[tt * 128:(tt + 1) * 128, :]
                        if le == 0:
                            nc.sync.dma_start(out=dst, in_=sc[:])
                        else:
                            nc.gpsimd.dma_start(out=dst, in_=sc[:],
                                                accum_op=Alu.add)

            # ---- cross-core reduce-scatter + shard output (fp32 out) ----
            nc.gpsimd.collective_compute(
                "ReduceScatter", Alu.add,
                replica_groups=[list(range(n_cores))],
                ins=[partial.ap().opt()],
                outs=[rs_out.ap().opt()],
            )
            with tc.tile_pool(name="cvt", bufs=2) as pcv:
                rw = min(128, tsh)
                for rb in range(0, tsh, rw):
                    rt_ = pcv.tile([rw, h], bf16, tag="rt_")
                    nc.sync.dma_start(out=rt_[:], in_=rs_out[rb:rb + rw, :])
                    cv = pcv.tile([rw, h], f32, tag="cv")
                    nc.vector.tensor_copy(out=cv[:], in_=rt_[:])
                    nc.sync.dma_start(out=out_sh[rb:rb + rw, :], in_=cv[:])

    nc.compile()
    return nc


def prep_inputs(x, gate_w, wv1, w2, t, h, f, e, n_cores):
    """Host-side shard/cast/tile. Returns per-core input maps."""
    import ml_dtypes
    bf16 = ml_dtypes.bfloat16

    epc = e // n_cores
    kh = h // 128
    kf = f // 128
    mf2 = 2 * f // 128

    xT = np.ascontiguousarray(x.T).astype(np.float32)        # [h, t]
    ident = np.eye(128, dtype=np.float32)

    in_maps = []
    for c in range(n_cores):
        own = list(range(c * epc, (c + 1) * epc))
        rest = [i for i in range(e) if i not in own]
        perm = own + rest
        gwT = np.ascontiguousarray(gate_w[perm].T).astype(np.float32)

        wl = wv1[own]                                        # [epc, 2f, h]
        # wv1t[le, k, m, hp, fp] = wv1[own[le], m*128+fp, k*128+hp]
        wv1tc = np.ascontiguousarray(
            wl.transpose(0, 2, 1)                            # [epc, h, 2f]
              .reshape(epc, kh, 128, mf2, 128)
              .transpose(0, 1, 3, 2, 4)).astype(bf16)

        w2l = w2[own]                                        # [epc, h, f]
        # w2t[le, k, fp, hh] = w2[own[le], hh, k*128+fp]
        w2tc = np.ascontiguousarray(
            w2l.transpose(0, 2, 1)                           # [epc, f, h]
               .reshape(epc, kf, 128, h)).astype(bf16)

        in_maps.append({
            "xT": xT,
            "gwT": gwT,
            "wv1t": wv1tc,
            "w2t": w2tc,
            "ident": ident,
        })
    return in_maps


def unshard(shards, t, h, n_cores):
    """Reassemble the full output from per-core RS shards (rank order)."""
    return np.concatenate(shards, axis=0).astype(np.float32)


def kernel(x, gate_w, wv1, w2, top_k):
    from concourse.bass_utils import run_bass_kernel_spmd

    assert int(top_k) == TOPK
    x = np.asarray(x, dtype=np.float32)
    gate_w = np.asarray(gate_w, dtype=np.float32)
    wv1 = np.asarray(wv1, dtype=np.float32)
    w2 = np.asarray(w2, dtype=np.float32)

    key = (T, H, F, E, NCORES)
    if key not in _CACHE:
        _CACHE[key] = build_moe_nc(T, H, F, E, NCORES, TOPK)
    nc = _CACHE[key]

    in_maps = prep_inputs(x, gate_w, wv1, w2, T, H, F, E, NCORES)
    res = run_bass_kernel_spmd(nc, in_maps, list(range(NCORES)))
    shards = [res.results[c]["out_shard"] for c in range(NCORES)]
    return unshard(shards, T, H, NCORES)
